# revision 4
# baseline (speedup 1.0000x reference)
"""AttGRU cell on 8 TRN2 NeuronCores.

Math (per reference):
    agg = einsum('ij,bj->bi', adj, x)                  # [B, N]
    r   = sigmoid(agg + h @ W_hr.T + b_hr)
    z   = sigmoid(agg + h @ W_hz.T + b_hz)
    n   = tanh(agg + r * (h @ W_hn.T + b_hn))
    out = (1 - z) * n + z * h
B=8, N=4096. Memory-bound: streaming the four [N, N] matrices dominates.

Sharding: row-shard adj/W_* over 8 cores (512 output features per core),
replicate x/h (tiny). Each core computes its 512 output columns; the host
concatenates. No collectives.

v3 design (all-fp8 stream, 8 MB/core; dual-queue DMA; warm PE):
- Every weight matrix ships as 1-byte fp8; per-gate formats sized to the
  error each term can carry (l2 rel err ~8.4e-3 vs the 2e-2 gate):
  * adj, W_hr: e4m3 (adj x4096, W_hr x64 so values land mid-range),
    consumed by DoubleRow matmuls (2 fp8 weights/PE cell -> one 1MB slab
    in 8 matmuls) with fp8 x/h stationaries. The r pre-activation error
    is attenuated by sigmoid'; agg is a ~0.01-std additive term.
  * W_hn, W_hz: e3m4 x128 (4-bit mantissa halves the quantization error
    vs e4m3; these set output accuracy directly), consumed by normal-mode
    matmuls with *bf16* h stationary (mixed-dtype matmul is legal; keeps
    the h quantization error out of the n/z gates).
- All 8 slab buffers live in SBUF simultaneously (8 MB of 24 MB): the
  HBM stream never waits on a buffer recycle.
- Each slab's two 8-chunk halves go to the two HWDGE rings (sync +
  scalar), so both queues pull concurrently and the stream runs at the
  HBM-per-core ceiling rather than single-queue packet rate.
- The PE idles ~11us during the Tile preamble + first-piece latency and
  would otherwise start HAM-throttled (1.2 GHz) and stay cold for the
  first ~11us of matmuls. A memset scratch tile + 8 dummy matmuls warm
  the clock gate to 8/8 before slab 0 lands.
- W_hz streams column-major (slab 6 = output cols 0-255 for all k,
  slab 7 = cols 256-511): the z accumulation for the first column half
  closes a full slab early, so its sigmoid/combine/out-DMA overlap the
  final slab's stream. The tail after the last weight byte is only the
  second half's quartered epilogue chain.
- Biases enter PSUM via K=1 matmuls (ones[1,B].T @ b[1,S]) as group
  openers; agg folds into the z accumulators via a 128*I f32 matmul.
  tanh(u) = 2*sigmoid(2u)-1 keeps ScalarE on a single activation table.

Per-core inputs (host-prepared):
  wdr  [4, 128, 16, 512] e4m3 - adj (slabs 0-1) | W_hr (slabs 2-3),
                                row-shard, transposed, chunk-major
  wnz  [4, 128, 8192] e3m4 - W_hn chunk-major (0-1) | W_hz col-major
                             (2-3), x128
  vtx8p [128, 2, 128] e4m3 - x.T chunk pairs (even plane 0 / odd plane 1)
  vth8p [128, 2, 128] e4m3 - h.T chunk pairs
  vth  [128, 256] bf16 - h.T per chunk (n/z-gate stationary)
  bvec [1, 1536] bf16  - b_hr*64 | b_hn*128 | b_hz*128 shards
  ones1 [1, 8] bf16, eyez [8, 8] f32 (=128*I), hloc [8, 512] f32
"""

from contextlib import ExitStack

import ml_dtypes
import numpy as np

import concourse.bass as bass
import concourse.tile as tile
from concourse import bacc, mybir
from concourse.bass_utils import run_bass_kernel_spmd

B = 8
N = 4096
NCORES = 8
S = N // NCORES          # 512 output cols per core
KC = 128                 # contraction chunk (PE partition dim)
NK = N // KC             # 32 chunks per gate
NKP = NK // 2            # 16 chunk pairs (DoubleRow)
CPS = 16                 # chunks per row-major slab ([128, 16, 512] = 1MB)
SLABW = CPS * S          # 8192 flat columns per slab
NSLABS = 8               # adj(2) + W_hr(2) + W_hn(2) + W_hz(2, col-major)
ZH = S // 2              # z-gate column half
ZQ = S // 4              # tail chain computed in column quarters
N_WARM = 8               # dummy matmuls to lift the PE clock gate
ADJ_SCALE = 4096.0       # adj pre-scale so e4m3 doesn't flush to zero
WHR_SCALE = 64.0         # W_hr pre-scale: N(0,1/64^2) -> N(0,1) for e4m3
WNZ_SCALE = 128.0        # W_hn/W_hz pre-scale for e3m4 (max normal 15.5)
USE_E3M4 = True          # e3m4 for W_hn/W_hz; False -> e4m3 fallback

BF16 = mybir.dt.bfloat16
F32 = mybir.dt.float32
FP8 = mybir.dt.float8e4
FP8E3 = mybir.dt.float8e3 if USE_E3M4 else mybir.dt.float8e4
DR = mybir.MatmulPerfMode.DoubleRow

_CACHED_NC = None


def _build():
    nc = bacc.Bacc(
        "TRN2",
        target_bir_lowering=False,
        debug=False,
        num_devices=NCORES,
    )
    wdr = nc.dram_tensor("wdr", [4, KC, CPS, S], FP8, kind="ExternalInput")
    wnz = nc.dram_tensor("wnz", [4, KC, SLABW], FP8E3, kind="ExternalInput")
    vtx8p = nc.dram_tensor("vtx8p", [KC, 2, NKP * B], FP8, kind="ExternalInput")
    vth8p = nc.dram_tensor("vth8p", [KC, 2, NKP * B], FP8, kind="ExternalInput")
    vth = nc.dram_tensor("vth", [KC, NK * B], BF16, kind="ExternalInput")
    bvec = nc.dram_tensor("bvec", [1, 3 * S], BF16, kind="ExternalInput")
    ones1 = nc.dram_tensor("ones1", [1, B], BF16, kind="ExternalInput")
    hloc = nc.dram_tensor("hloc", [B, S], F32, kind="ExternalInput")
    eyez = nc.dram_tensor("eyez", [B, B], F32, kind="ExternalInput")
    out = nc.dram_tensor("out", [B, S], F32, kind="ExternalOutput")

    AF = mybir.ActivationFunctionType
    ALU = mybir.AluOpType

    with tile.TileContext(nc) as tc, ExitStack() as ctx:
        wpool = ctx.enter_context(tc.tile_pool(name="wall", bufs=NSLABS))
        cpool = ctx.enter_context(tc.tile_pool(name="const", bufs=1))
        ppool = ctx.enter_context(tc.tile_pool(name="acc", bufs=1, space="PSUM"))
        epool = ctx.enter_context(tc.tile_pool(name="epi", bufs=1))

        # PE warmup: the PE sits idle through the Tile preamble while the
        # first slab is in flight, so its HAM clock gate is at 4/8 when
        # real matmuls start and stays cold for ~3.4us of busy time. Spin
        # ~8 dummy matmuls on a memset tile (no DMA dependency) to cross
        # the busy window before slab 0's data lands.
        warm_sb = cpool.tile([KC, S], BF16, tag="warm")
        nc.vector.memset(warm_sb[:], 0.0)
        warm_ps = ppool.tile([B, S], F32, tag="warmps")
        for _ in range(N_WARM):
            nc.tensor.matmul(
                warm_ps[:, :], warm_sb[:, :B], warm_sb[:, :], start=True, stop=True
            )

        # all consts on gpsimd SWDGE (vtx8p first - the first matmul needs
        # it); the HWDGE rings stay clear so slab 0 issues immediately
        vtx8p_sb = cpool.tile([KC, 2, NKP * B], FP8, tag="vtx8p")
        nc.gpsimd.dma_start(vtx8p_sb[:], vtx8p[:])
        vth8p_sb = cpool.tile([KC, 2, NKP * B], FP8, tag="vth8p")
        nc.gpsimd.dma_start(vth8p_sb[:], vth8p[:])
        vth_sb = cpool.tile([KC, NK * B], BF16, tag="vth")
        nc.gpsimd.dma_start(vth_sb[:], vth[:])
        bvec_sb = cpool.tile([1, 3 * S], BF16, tag="bvec")
        nc.gpsimd.dma_start(bvec_sb[:], bvec[:])
        ones_sb = cpool.tile([1, B], BF16, tag="ones1")
        nc.gpsimd.dma_start(ones_sb[:], ones1[:])
        hloc_sb = cpool.tile([B, S], F32, tag="hloc")
        nc.gpsimd.dma_start(hloc_sb[:], hloc[:])
        eyez_sb = cpool.tile([B, B], F32, tag="eyez")
        nc.gpsimd.dma_start(eyez_sb[:], eyez[:])

        acc = [
            ppool.tile([B, S], F32, tag=f"acc{g}", name=f"acc{g}") for g in range(3)
        ]
        accz = [
            ppool.tile([B, ZH], F32, tag=f"accz{i}", name=f"accz{i}")
            for i in range(2)
        ]

        # epilogue tiles, declared up front
        s_agg = epool.tile([B, S], F32, tag="sagg")
        t_r = epool.tile([B, S], F32, tag="tr")
        r_t = epool.tile([B, S], F32, tag="r")
        t_n = epool.tile([B, S], F32, tag="tn")
        t_n2 = epool.tile([B, S], F32, tag="tn2")
        sg_t = epool.tile([B, S], F32, tag="sg")
        n_t = epool.tile([B, S], F32, tag="n")
        d_t = epool.tile([B, S], F32, tag="d")
        z_t = epool.tile([B, S], F32, tag="z")
        zd_t = epool.tile([B, S], F32, tag="zd")
        o_t = epool.tile([B, S], F32, tag="o")

        def bias_open(ps, lo, width):
            # psum = ones[1,B].T @ b[1,width]: broadcasts the bias into a
            # freshly-cleared accumulator
            nc.tensor.matmul(
                ps, ones_sb[:, :], bvec_sb[:, lo : lo + width],
                start=True, stop=False,
            )

        def slab_dma(wa_flat, src_flat, first, last):
            # halves of every slab ride different HWDGE rings so the two
            # queues pull from HBM concurrently; small leading pieces on
            # slab 0 start the PE early, small trailing pieces on the
            # last slab cut the PE's wait on the final transfer
            if first:
                sync_splits, scalar_splits = (1024, 1024, 2048), (4096,)
            elif last:
                sync_splits, scalar_splits = (4096,), (2048, 2048)
            else:
                sync_splits, scalar_splits = (4096,), (4096,)
            c0 = 0
            for w in sync_splits:
                nc.sync.dma_start(wa_flat[:, c0 : c0 + w], src_flat[:, c0 : c0 + w])
                c0 += w
            for w in scalar_splits:
                nc.scalar.dma_start(
                    wa_flat[:, c0 : c0 + w], src_flat[:, c0 : c0 + w]
                )
                c0 += w

        for sl in range(NSLABS):
            g, half = divmod(sl, 2)
            if g < 2:
                # DoubleRow fp8 gates: adj (g=0), W_hr (g=1)
                wa = wpool.tile([KC, CPS, S], FP8, tag="wa", name=f"wa{sl}")
                slab_dma(
                    wa.rearrange("p c s -> p (c s)"),
                    wdr[sl].rearrange("p c s -> p (c s)"),
                    first=(sl == 0), last=False,
                )
                vp = vtx8p_sb if g == 0 else vth8p_sb
                for c in range(0, CPS, 2):
                    kp = half * (CPS // 2) + c // 2
                    if g == 1 and kp == 0:
                        bias_open(acc[1][:, :], 0, S)
                    nc.tensor.matmul(
                        acc[g][:, :],
                        vp[:, :, kp * B : (kp + 1) * B],
                        wa[:, c : c + 2, :],
                        start=(g == 0 and kp == 0),
                        stop=(kp == NKP - 1),
                        perf_mode=DR,
                    )
                    if kp != NKP - 1:
                        continue
                    if g == 0:
                        # descale agg (adj was pre-scaled for e4m3 range)
                        nc.vector.tensor_scalar_mul(
                            s_agg[:], acc[0][:, :], 1.0 / ADJ_SCALE
                        )
                    else:
                        # t_r = acc1/WHR_SCALE + agg, then sigmoid
                        nc.vector.scalar_tensor_tensor(
                            t_r[:], acc[1][:, :], 1.0 / WHR_SCALE, s_agg[:],
                            ALU.mult, ALU.add,
                        )
                        nc.scalar.activation(r_t[:], t_r[:], AF.Sigmoid)
            elif g == 2:
                # W_hn, chunk-major, e3m4 moving x bf16 h stationary
                wa = wpool.tile([KC, SLABW], FP8E3, tag="wa", name=f"wa{sl}")
                slab_dma(wa, wnz[sl - 4], first=False, last=False)
                for c in range(CPS):
                    k = half * CPS + c
                    if k == 0:
                        bias_open(acc[2][:, :], S, S)
                    nc.tensor.matmul(
                        acc[2][:, :],
                        vth_sb[:, k * B : (k + 1) * B],
                        wa[:, c * S : (c + 1) * S],
                        start=False,
                        stop=(k == NK - 1),
                    )
                    if k != NK - 1:
                        continue
                    # n epilogue overlaps the W_hz stream
                    nc.vector.tensor_mul(t_n[:], acc[2][:, :], r_t[:])
                    nc.vector.scalar_tensor_tensor(
                        t_n2[:], t_n[:], 1.0 / WNZ_SCALE, s_agg[:],
                        ALU.mult, ALU.add,
                    )
                    # tanh(u) = 2*sigmoid(2u) - 1 (ACT on one table)
                    nc.scalar.activation(sg_t[:], t_n2[:], AF.Sigmoid, scale=2.0)
                    nc.vector.tensor_scalar(
                        n_t[:], sg_t[:], 2.0, 1.0, ALU.mult, ALU.subtract
                    )
                    nc.vector.tensor_sub(d_t[:], hloc_sb[:], n_t[:])
            else:
                # W_hz, column-major: slab covers output cols
                # [half*256, half*256+256) for all 32 k-chunks
                hf = half
                zlo = hf * ZH
                wa = wpool.tile([KC, SLABW], FP8E3, tag="wa", name=f"wa{sl}")
                slab_dma(wa, wnz[sl - 4], first=False, last=(sl == NSLABS - 1))
                ps = accz[hf][:, :]
                bias_open(ps, 2 * S + zlo, ZH)
                # fold WNZ_SCALE*agg into this half's z accumulator
                nc.tensor.matmul(
                    ps, eyez_sb[:, :], s_agg[:, zlo : zlo + ZH],
                    start=False, stop=False,
                )
                for k in range(NK):
                    nc.tensor.matmul(
                        ps,
                        vth_sb[:, k * B : (k + 1) * B],
                        wa[:, k * ZH : (k + 1) * ZH],
                        start=False,
                        stop=(k == NK - 1),
                    )
                # z tail for this half, quartered: pipelines ACT/DVE with
                # the out-DMA dispatches; the first half's whole chain
                # overlaps the second half's slab stream
                for q in range(2):
                    cols = slice(zlo + q * ZQ, zlo + (q + 1) * ZQ)
                    nc.scalar.activation(
                        z_t[:, cols], accz[hf][:, q * ZQ : (q + 1) * ZQ],
                        AF.Sigmoid, scale=1.0 / WNZ_SCALE,
                    )
                    nc.vector.tensor_mul(
                        zd_t[:, cols], z_t[:, cols], d_t[:, cols]
                    )
                    nc.vector.tensor_add(
                        o_t[:, cols], zd_t[:, cols], n_t[:, cols]
                    )
                    dma_eng = nc.sync if hf == 0 else nc.scalar
                    dma_eng.dma_start(out[:, cols], o_t[:, cols])

    nc.compile()
    return nc


def _get_nc():
    global _CACHED_NC
    if _CACHED_NC is None:
        _CACHED_NC = _build()
    return _CACHED_NC


def make_in_maps(x, h, adj, W_hr, b_hr, W_hz, b_hz, W_hn, b_hn):
    bf = ml_dtypes.bfloat16
    fp8 = ml_dtypes.float8_e4m3fn
    fp8e3 = ml_dtypes.float8_e3m4 if USE_E3M4 else ml_dtypes.float8_e4m3fn
    x = np.asarray(x, np.float32)
    h = np.asarray(h, np.float32)
    adj = np.asarray(adj, np.float32)
    W_hr = np.asarray(W_hr, np.float32)
    W_hz = np.asarray(W_hz, np.float32)
    W_hn = np.asarray(W_hn, np.float32)
    b_hr = np.asarray(b_hr, np.float32)
    b_hz = np.asarray(b_hz, np.float32)
    b_hn = np.asarray(b_hn, np.float32)

    def pack_vt(v):
        # [B, N] -> [KC, NK, B] chunk-major
        return np.ascontiguousarray(v.T.reshape(NK, KC, B).transpose(1, 0, 2))

    def pack_vt_pairs(v):
        # [KC, NK, B] -> [KC, 2, NKP*B]: even chunks plane 0, odd plane 1
        c = pack_vt(v)
        return np.ascontiguousarray(
            c.reshape(KC, NKP, 2, B).transpose(0, 2, 1, 3).reshape(KC, 2, NKP * B)
        )

    vtx8p_packed = pack_vt_pairs(x).astype(fp8)
    vth8p_packed = pack_vt_pairs(h).astype(fp8)
    vth_packed = pack_vt(h).reshape(KC, NK * B).astype(bf)

    def pack_slabs(chunks_2d, nslabs):
        # [N, W] (contraction-major) -> [nslabs, KC, CPS, W]
        w = chunks_2d.shape[1]
        return np.ascontiguousarray(
            chunks_2d.reshape(nslabs, CPS, KC, w).transpose(0, 2, 1, 3)
        )

    def pack_colmajor(mT):
        # [N, S] -> [2, KC, NK, ZH]: slab h = output cols [h*ZH,(h+1)*ZH)
        # for all NK k-chunks
        r = mT.reshape(NK, KC, S)
        return np.ascontiguousarray(
            np.stack([r[:, :, :ZH], r[:, :, ZH:]]).transpose(0, 2, 1, 3)
        )

    in_maps = []
    for s in range(NCORES):
        rs, re = s * S, (s + 1) * S
        wdrp = np.concatenate(
            [
                pack_slabs(np.ascontiguousarray(adj[rs:re].T) * ADJ_SCALE, 2),
                pack_slabs(np.ascontiguousarray(W_hr[rs:re].T) * WHR_SCALE, 2),
            ]
        ).astype(fp8)
        whn_s = pack_slabs(
            np.clip(W_hn[rs:re].T * WNZ_SCALE, -15.0, 15.0), 2
        ).reshape(2, KC, SLABW)
        whz_s = pack_colmajor(
            np.clip(W_hz[rs:re].T * WNZ_SCALE, -15.0, 15.0)
        ).reshape(2, KC, SLABW)
        wnzp = np.concatenate([whn_s, whz_s]).astype(fp8e3)
        bvecp = np.concatenate(
            [b_hr[rs:re] * WHR_SCALE, b_hn[rs:re] * WNZ_SCALE,
             b_hz[rs:re] * WNZ_SCALE]
        )[None, :].astype(bf)
        in_maps.append(
            {
                "wdr": wdrp,
                "wnz": wnzp,
                "vtx8p": vtx8p_packed,
                "vth8p": vth8p_packed,
                "vth": vth_packed,
                "bvec": bvecp,
                "ones1": np.ones((1, B), dtype=bf),
                "hloc": np.ascontiguousarray(h[:, rs:re]),
                "eyez": np.eye(B, dtype=np.float32) * WNZ_SCALE,
            }
        )
    return in_maps


def run(in_maps, trace=False, **kw):
    nc = _get_nc()
    return run_bass_kernel_spmd(
        nc, in_maps, core_ids=list(range(NCORES)), trace=trace, **kw
    )


def kernel(x, h, adj, W_hr, b_hr, W_hz, b_hz, W_hn, b_hn):
    in_maps = make_in_maps(x, h, adj, W_hr, b_hr, W_hz, b_hz, W_hn, b_hn)
    res = run(in_maps)
    return np.concatenate(
        [np.asarray(res.results[s]["out"]) for s in range(NCORES)], axis=1
    )
